# revision 1
# baseline (speedup 1.0000x reference)
# Trainium2 Bass kernel for nn_Attention_88313117540497.
#
# Reference computation (per batch b of 128):
#   v = x_b @ Wv                      (196, 384) @ (384, 512)
#   conv: each of the 512 channels' 14x14 image convolved with a 27x27
#         kernel qk at padding 13 -> same 14x14 output
#   y = conv_out @ Wo + bo            (196, 512) @ (512, 384)
#
# Key observations:
#  1. A 27x27 kernel on a 14x14 image with padding 13 covers every input
#     pixel for every output pixel, so the conv is exactly a dense linear
#     map over the 196 positions: out[p] = sum_u M[p, u] * img[u] with
#     M[(p,q),(u,v)] = qk[13+u-p, 13+v-q]. M is shared across all
#     batches and channels, so conv == matmul with a 196x196 matrix.
#  2. The whole module is then  y_b = M @ x_b @ Wv @ Wo + bo.  Folding
#     W = Wv @ Wo (384x384, computed once on device) removes the
#     INNER=512 dimension entirely: y_b = M @ (x_b @ W) + bo, which
#     halves the FLOPs.
#  3. Feeding x transposed (features major) makes both matmuls natural
#     for the PE (contraction dim on partitions for both operands, no
#     on-device transposes):
#        H_b = XT_b.T @ W      (lhsT = XT_b, rhs = W)   -> (196, 384)
#        Y_b = MT.T @ H_b      (lhsT = MT,   rhs = H_b) -> (196, 384)
#     with MT = M^T. All matmuls have free dim N = 384 >= 256, so
#     float32r runs at 1 cycle/row (4x faster than plain fp32).
#
# Sharding: data-parallel over batch, 16 batches per core, weights
# replicated. No collectives needed.
#
# DMA layout (from trace analysis): dma_start triggers cost ~0.65us
# each on the issuing sequencer and every completion semaphore pays a
# ~2us HBM write-receipt, so transfers are few and large. Reads spread
# across all 16 SDMA engines, but SBUF->HBM writes are pinned to 4
# queues (~115GB/s), with the SWDGE path adding independent write
# lanes. So: x is host-packed feature-major (12.5KB DRAM rows), loaded
# as one DMA per 4-batch group on the sync sequencer (pure prefetch
# stream, no data-dependent stalls); y is written in a PE-native
# k-major grouped layout (host-reassembled) with the 128-partition
# chunk streamed per 2 batches on HWDGE and the 68-partition chunk per
# 4-batch group on SWDGE; small constants ride SWDGE early. Tiny fp32
# const-AP matmuls warm the PE clock before the first data lands.

import numpy as np

import concourse.bass as bass
from concourse import bacc
import concourse.mybir as mybir
import concourse.tile as tile
from concourse.bass_utils import run_bass_kernel_spmd

N_CORES = 8
B = 128                 # total batch
BPC = B // N_CORES      # batches per core
DIM = 384
INNER = 512
NPOS = 196              # 14*14 positions
IMG = 14
KS = 27                 # conv kernel size

F32 = mybir.dt.float32
F32R = mybir.dt.float32r

TOK_CHUNKS = [(0, 128), (128, 68)]
DCH = DIM // 128        # 3 feature chunks (contraction of stage 1)
ICH = INNER // 128      # 4 inner chunks (contraction of the fold)
# progressive X-load groups: a small first group lands quickly so the
# PE stream is gapless from first data; later groups amortize triggers
XGROUPS = [(0, 2), (2, 6), (8, 8)]   # (start batch, size)
GXMAX = max(sz for _, sz in XGROUPS)
GY = 4                  # batches per Y-store group
NGY = BPC // GY

# float32r (= tfloat32) runs at full PE rate for free dim >= 256. The
# BIR verifier requires producers of f32r matmul operands to write
# pre-rounded TF32: DMA-fed operands are rounded on the host, on-chip
# producers (PSUM evictions) write float32r directly.
MM_DT = F32R


def build_program():
    nc = bacc.Bacc("TRN2", debug=False)

    # x, feature-major: [feature chunk, partition (feature%128), token]
    xt_d = nc.dram_tensor("xt", [DCH, 128, BPC * NPOS], MM_DT,
                          kind="ExternalInput")
    w_d = nc.dram_tensor("w", [DIM, DIM], MM_DT, kind="ExternalInput")
    mt_d = nc.dram_tensor("mt", [NPOS, NPOS], MM_DT, kind="ExternalInput")
    bias_d = nc.dram_tensor("bias", [128, DIM], MM_DT, kind="ExternalInput")
    # y, PE-native: [group, p-chunk k, partition, batch-in-group, e]
    y_d = nc.dram_tensor("y", [NGY, 2, 128, GY, DIM], F32,
                         kind="ExternalOutput")

    GTMAX = GXMAX * NPOS
    xgrp = {}            # batch -> (start, size) of its group
    for s0, sz in XGROUPS:
        for bb in range(s0, s0 + sz):
            xgrp[bb] = (s0, sz)

    with tile.TileContext(nc) as tc:
        with (
            tc.tile_pool(name="const", bufs=1) as const,
            tc.tile_pool(name="work", bufs=2) as work,
            tc.tile_pool(name="psum", bufs=2, space="PSUM") as psum,
        ):
            dges = [nc.sync, nc.scalar]

            # ---- small constants via SWDGE (keeps HWDGE queues free) ----
            bias_sb = const.tile([128, DIM], MM_DT)
            nc.gpsimd.dma_start(bias_sb[:, :], bias_d[:, :])
            mt_sb = const.tile([128, 2 * NPOS], MM_DT)
            for uc, (u0, usz) in enumerate(TOK_CHUNKS):
                nc.gpsimd.dma_start(
                    mt_sb[:usz, uc * NPOS:(uc + 1) * NPOS],
                    mt_d[u0:u0 + usz, :],
                )

            # ---- folded weight W = Wv @ Wo (host-precomputed) ----
            w_sb = const.tile([128, DCH * DIM], MM_DT)
            nc.scalar.dma_start(
                w_sb[:, :].rearrange("p (c e) -> p c e", c=DCH),
                w_d.rearrange("(c p) e -> p c e", p=128),
            )

            # ---- PE warm-up on framework const APs (ready right after
            # instruction load, no DMA dependency): tiny fp32 matmuls keep
            # the tensor engine busy so the clock is ramped when the
            # stage-1 stream begins ----
            warm_c = nc.const_aps.tensor(1.0, (128, 1))
            for wi in range(28):
                warm = psum.tile([128, DIM], F32, tag="y1", name=f"warm{wi}")
                nc.tensor.matmul(
                    warm[0:1, 0:1],
                    lhsT=warm_c,
                    rhs=warm_c,
                    start=True,
                    stop=True,
                )

            # ---- main loop ----
            xt_t = None
            y_t = None
            for b in range(BPC):
                gstart, gsize = xgrp[b]
                gt = gsize * NPOS
                if b == gstart:
                    xt_t = work.tile([128, DCH * GTMAX], MM_DT, tag="xt",
                                     bufs=3, name=f"xt{gstart}")
                    ts0, ts1 = gstart * NPOS, (gstart + gsize) * NPOS
                    if gstart == 0:
                        # first group per feature chunk so stage-1 can start
                        # on chunk 0 while chunks 1-2 are in flight
                        for c in range(DCH):
                            nc.sync.dma_start(
                                xt_t[:, c * gt:(c + 1) * gt],
                                xt_d[c, :, ts0:ts1],
                            )
                    else:
                        nc.sync.dma_start(
                            xt_t[:, 0:DCH * gt].rearrange(
                                "p (c t) -> p c t", c=DCH),
                            xt_d[:, :, ts0:ts1].rearrange("c p t -> p c t"),
                        )
                if b % GY == 0:
                    # [k-chunk, batch-in-group, e] per partition
                    y_t = work.tile([128, 2 * GY * DIM], F32, tag="y", bufs=3,
                                    name=f"y{b // GY}")

                tok0 = (b - gstart) * NPOS
                bi = b % GY

                # stage 1: H_b = XT_b.T @ W  (tokens on partitions)
                h_t = work.tile([128, 2 * DIM], MM_DT, tag="h", bufs=4,
                                name=f"h{b}")
                for t, (u0, usz) in enumerate(TOK_CHUNKS):
                    ph = psum.tile([128, DIM], F32, tag=f"h{t}", name=f"ph{t}_{b}")
                    for c in range(DCH):
                        o = c * gt + tok0 + u0
                        nc.tensor.matmul(
                            ph[:usz, :],
                            lhsT=xt_t[:, o:o + usz],
                            rhs=w_sb[:, c * DIM:(c + 1) * DIM],
                            start=(c == 0),
                            stop=(c == DCH - 1),
                        )
                    nc.scalar.copy(h_t[:usz, t * DIM:(t + 1) * DIM],
                                   ph[:usz, :])

                # stage 2: Y_b = MT.T @ H_b + bias
                for t2, (p0, psz) in enumerate(TOK_CHUNKS):
                    py = psum.tile([128, DIM], F32, tag=f"y{t2}", name=f"py{t2}_{b}")
                    for uc, (u0, usz) in enumerate(TOK_CHUNKS):
                        nc.tensor.matmul(
                            py[:psz, :],
                            lhsT=mt_sb[:usz, uc * NPOS + p0:uc * NPOS + p0 + psz],
                            rhs=h_t[:usz, uc * DIM:(uc + 1) * DIM],
                            start=(uc == 0),
                            stop=(uc == 1),
                        )
                    nc.vector.tensor_add(
                        y_t[:psz, t2 * GY * DIM + bi * DIM:
                            t2 * GY * DIM + (bi + 1) * DIM],
                        py[:psz, :],
                        bias_sb[:psz, :].bitcast(F32),
                    )

                g = b // GY
                # stream k0 out per 2 batches on the (4-queue) HWDGE write
                # path so it is busy as early as possible; k1 rides the
                # otherwise-idle SWDGE path. The last group flushes in the
                # finest grains so the final transfer (and its completion
                # receipt) is as small as possible.
                last_group = (gstart + gsize == BPC)
                if last_group and b >= BPC - 2:
                    bi2 = b % GY
                    nc.sync.dma_start(
                        y_d[g, 0, :, bi2:bi2 + 1],
                        y_t[:, bi2 * DIM:(bi2 + 1) * DIM])
                elif b % 2 == 1:
                    h2 = (b % GY) // 2
                    nc.sync.dma_start(
                        y_d[g, 0, :, 2 * h2:2 * h2 + 2],
                        y_t[:, 2 * h2 * DIM:(2 * h2 + 2) * DIM])
                if last_group and b % 2 == 1:
                    h2 = (b % GY) // 2
                    nc.gpsimd.dma_start(
                        y_d[g, 1, 0:68, 2 * h2:2 * h2 + 2],
                        y_t[:68, (GY + 2 * h2) * DIM:(GY + 2 * h2 + 2) * DIM])
                elif b % GY == GY - 1:
                    nc.gpsimd.dma_start(
                        y_d[g, 1, 0:68], y_t[:68, GY * DIM:2 * GY * DIM])

    nc.compile()
    return nc


_PROGRAM = None


def _get_program():
    global _PROGRAM
    if _PROGRAM is None:
        _PROGRAM = build_program()
    return _PROGRAM


def _round_tf32(a):
    # round-to-nearest to the 10-bit TF32 mantissa (dtype-format conversion
    # for the float32r DRAM tensors)
    b = (a.view(np.uint32) + np.uint32(0x1000)) & np.uint32(0xFFFFE000)
    return b.view(np.float32)


def _host_prep(x, Wv, qk, Wo, bo):
    x = np.asarray(x, dtype=np.float32)
    # per-core feature-major token stream: (cores, 3, 128, BPC*196)
    XTC = np.ascontiguousarray(
        x.reshape(N_CORES, BPC * NPOS, DIM).transpose(0, 2, 1)
    ).reshape(N_CORES, DCH, 128, BPC * NPOS)
    XTC = _round_tf32(XTC)
    # one-time weight prep: fold the two projections (fp32 matmul), then
    # round to TF32 for the float32r stage-1 weights
    W = _round_tf32(np.ascontiguousarray(
        np.asarray(Wv, np.float32) @ np.asarray(Wo, np.float32)))
    # MT[(u,v),(p,q)] = qk[13+u-p, 13+v-q]  (pure gather, no arithmetic)
    qk2 = np.asarray(qk, np.float32).reshape(KS, KS)
    idx = (KS // 2) + np.arange(IMG)[:, None] - np.arange(IMG)[None, :]
    MT = _round_tf32(np.ascontiguousarray(
        qk2[idx[:, None, :, None], idx[None, :, None, :]].reshape(NPOS, NPOS)
    ))
    bias = np.ascontiguousarray(
        np.broadcast_to(np.asarray(bo, np.float32), (128, DIM))
    )
    return XTC, W, MT, bias


def _unpack_core(y2):
    # y2: [NGY, 2, 128, GY, DIM] -> (BPC, NPOS, DIM)
    out = np.empty((BPC, NPOS, DIM), np.float32)
    top = y2[:, 0].transpose(0, 2, 1, 3)          # [NGY, GY, 128, DIM]
    bot = y2[:, 1, 0:68].transpose(0, 2, 1, 3)    # [NGY, GY, 68, DIM]
    out[:, 0:128, :] = top.reshape(BPC, 128, DIM)
    out[:, 128:NPOS, :] = bot.reshape(BPC, 68, DIM)
    return out


def _run(x, Wv, qk, Wo, bo, **spmd_kwargs):
    XTC, W, MT, bias = _host_prep(x, Wv, qk, Wo, bo)
    nc = _get_program()
    in_maps = [
        {"xt": XTC[c], "w": W, "mt": MT, "bias": bias}
        for c in range(N_CORES)
    ]
    res = run_bass_kernel_spmd(nc, in_maps, list(range(N_CORES)), **spmd_kwargs)
    y = np.concatenate(
        [_unpack_core(res.results[c]["y"]) for c in range(N_CORES)], axis=0)
    return y, res


def kernel(x, Wv, qk, Wo, bo):
    y, _ = _run(x, Wv, qk, Wo, bo)
    return y



# revision 5
# speedup vs baseline: 1.0967x; 1.0967x over previous
# Trainium2 Bass kernel for nn_Attention_88313117540497.
#
# Reference computation (per batch b of 128):
#   v = x_b @ Wv                      (196, 384) @ (384, 512)
#   conv: each of the 512 channels' 14x14 image convolved with a 27x27
#         kernel qk at padding 13 -> same 14x14 output
#   y = conv_out @ Wo + bo            (196, 512) @ (512, 384)
#
# Math restructuring:
#  1. The 27x27 kernel at padding 13 covers every input pixel for every
#     output pixel, so the conv is a dense 196x196 linear map M over
#     positions, shared by all batches/channels: conv == matmul.
#  2. Folding W = Wv @ Wo (384x384) removes INNER=512:
#     y_b = M @ (x_b @ W) + bo = (M @ x_b) @ W + bo.
#  3. Dataflow (all-transposed, M-first) minimizes PE streaming cycles:
#       Z.T = lhsT(X_b).T-free @ rhs(MT):  Z.T[d,p] = sum_u X[u,d] MT[u,p]
#         out: 3 d-chunks x 196 cols, contraction u=196 (2 chunks)
#         -> 6 matmuls/batch of N=196
#       Y.T = lhsT(W).T @ rhs(Z.T):        Y.T[e,p] = sum_d W[d,e] Z.T[d,p]
#         out: 3 e-chunks, contraction d=384 (3 exact chunks), rhs spans
#         2 batches -> 9 matmuls per 2-batch group of N=392
#     2940 PE cycles/batch vs 3840 for the W-first token-major order, and
#     the awkward 68-row token chunk appears only as a contraction chunk.
#  4. Everything bf16 (validated 4.4e-3 max-normalized error vs 2e-2
#     budget): 1 cycle/row at any N (f32r drops to 4 cyc/row under
#     N=256), FWL double-rate LDWEIGHTS (fp32-family needs 2 full-rate
#     weight loads per matmul - the baseline's PE throttle), and half
#     the HBM traffic in both directions.
#
# Sharding: data-parallel over batch, 16 batches per core, weights
# replicated. No collectives.
#
# Engine budget per core (theory): PE ~21us of matmul stream; PSUM
# evictions ~1.45ns/elem/partition split vector/scalar ~14us each;
# x loads 2.4MB + y stores 2.4MB bf16 spread over sync HWDGE, scalar
# HWDGE and gpsimd SWDGE rings, all overlapped with compute. Small
# final stores keep the end-of-kernel write receipt off the critical
# path. Const-AP warmup matmuls ramp the PE clock before data lands.

import numpy as np
import ml_dtypes

import concourse.bass as bass
from concourse import bacc
import concourse.mybir as mybir
import concourse.tile as tile
from concourse.bass_utils import run_bass_kernel_spmd

N_CORES = 8
B = 128                 # total batch
BPC = B // N_CORES      # batches per core
DIM = 384
NPOS = 196              # 14*14 positions
IMG = 14
KS = 27                 # conv kernel size
U0 = 128                # token contraction chunk 0
U1 = NPOS - U0          # token contraction chunk 1 (68)

F32 = mybir.dt.float32
BF16 = mybir.dt.bfloat16
NP_BF16 = ml_dtypes.bfloat16

NG = BPC // 2           # 2-batch compute groups
GW = 2 * NPOS           # tokens (= output cols) per group: 392
# progressive x-load groups (start batch, count): small first group so
# the PE stream starts as early as possible
XGROUPS = [(0, 2), (2, 2), (4, 4), (8, 4), (12, 4)]
NXG = len(XGROUPS)


def build_program():
    nc = bacc.Bacc("TRN2", debug=False)

    # x split by token chunk, natural token-major layout (no host transpose)
    xu0_d = nc.dram_tensor("xu0", [BPC, U0, DIM], BF16, kind="ExternalInput")
    xu1_d = nc.dram_tensor("xu1", [BPC, U1, DIM], BF16, kind="ExternalInput")
    # MT packed: cols 0:196 = MT[0:128,:] ; cols 196:392 rows 0:68 = MT[128:196,:]
    mt_d = nc.dram_tensor("mt", [U0, GW], BF16, kind="ExternalInput")
    # W folded, tiled: block k*3+j = W[128k:128k+128, 128j:128j+128]
    w_d = nc.dram_tensor("w", [128, 9 * 128], BF16, kind="ExternalInput")
    bias_d = nc.dram_tensor("bias", [128, 3], F32, kind="ExternalInput")
    # y transposed: [e-chunk, e%128, batch-token stream]
    y_d = nc.dram_tensor("y", [3, 128, BPC * NPOS], BF16, kind="ExternalOutput")

    with tile.TileContext(nc) as tc:
        with (
            tc.tile_pool(name="const", bufs=1) as const,
            tc.tile_pool(name="work", bufs=2) as work,
            tc.tile_pool(name="psum", bufs=2, space="PSUM") as psum,
        ):
            # ---- constants ----
            mt_sb = const.tile([U0, GW], BF16)
            nc.gpsimd.dma_start(mt_sb[:, :], mt_d[:, :])
            bias_sb = const.tile([128, 3], F32)
            nc.gpsimd.dma_start(bias_sb[:, :], bias_d[:, :])
            w_sb = const.tile([128, 9 * 128], BF16)
            nc.scalar.dma_start(w_sb[:, :], w_d[:, :])

            # ---- x loads: sync HWDGE for the first 3 groups, SWDGE for
            # the back half (keeps the sync ring free for y stores) ----
            xu0_t = {}
            xu1_t = {}
            for gi, (s, nb) in enumerate(XGROUPS):
                eng = nc.sync if gi < 3 else nc.gpsimd
                t0 = work.tile([U0, nb * DIM], BF16, tag="xu0", bufs=NXG,
                               name=f"xu0_{gi}")
                eng.dma_start(
                    t0[:, 0:nb * DIM].rearrange("p (b d) -> p b d", b=nb),
                    xu0_d[s:s + nb, :, :].rearrange("b p d -> p b d"),
                )
                t1 = work.tile([U1, nb * DIM], BF16, tag="xu1", bufs=NXG,
                               name=f"xu1_{gi}")
                eng.dma_start(
                    t1[:, 0:nb * DIM].rearrange("p (b d) -> p b d", b=nb),
                    xu1_d[s:s + nb, :, :].rearrange("b p d -> p b d"),
                )
                for b in range(s, s + nb):
                    xu0_t[b] = (t0, (b - s) * DIM)
                    xu1_t[b] = (t1, (b - s) * DIM)

            # ---- PE warm-up on framework const APs: ramp the HAM clock
            # while the first data is in flight ----
            warm_c = nc.const_aps.tensor(1.0, (128, 1))
            for wi in range(10):
                warm = psum.tile([128, GW], F32, tag="z0", name=f"warm{wi}")
                nc.tensor.matmul(
                    warm[0:1, 0:1], lhsT=warm_c, rhs=warm_c,
                    start=True, stop=True,
                )

            # ---- main loop: 2-batch groups ----
            ZBUFS = [2, 2, 1]       # PSUM slots per z tag (total 5)
            ysb = {}
            for g in range(NG):
                ba, bb = 2 * g, 2 * g + 1
                # stage 1: Z.T chunks
                zsb = []
                for k in range(3):
                    zp = psum.tile([128, GW], F32, tag=f"z{k}",
                                   bufs=ZBUFS[k], name=f"zp{k}_{g}")
                    for half, b in ((0, ba), (1, bb)):
                        t0, off0 = xu0_t[b]
                        t1, off1 = xu1_t[b]
                        c0 = half * NPOS
                        nc.tensor.matmul(
                            zp[:, c0:c0 + NPOS],
                            lhsT=t0[:, off0 + k * 128:off0 + (k + 1) * 128],
                            rhs=mt_sb[:, 0:NPOS],
                            start=True, stop=False,
                        )
                        nc.tensor.matmul(
                            zp[:, c0:c0 + NPOS],
                            lhsT=t1[0:U1, off1 + k * 128:off1 + (k + 1) * 128],
                            rhs=mt_sb[0:U1, NPOS:GW],
                            start=False, stop=True,
                        )
                    z = work.tile([128, GW], BF16, tag=f"zsb{k}", bufs=2,
                                  name=f"zsb{k}_{g}")
                    # z evictions on scalar (ACT copy with fp32->bf16 cast)
                    nc.scalar.copy(z[:, :], zp[:, :])
                    zsb.append(z)

                # stage 2: Y.T chunks (+bias, cast to bf16)
                pair, half = g // 2, g % 2
                last2 = g >= NG - 2
                for j in range(3):
                    yp = psum.tile([128, GW], F32, tag=f"y{j}", bufs=1,
                                   name=f"yp{j}_{g}")
                    for k in range(3):
                        nc.tensor.matmul(
                            yp[:, :],
                            lhsT=w_sb[:, (k * 3 + j) * 128:(k * 3 + j + 1) * 128],
                            rhs=zsb[k][:, :],
                            start=(k == 0), stop=(k == 2),
                        )
                    if last2:
                        yt = work.tile([128, GW], BF16, tag=f"ysb{j}", bufs=2,
                                       name=f"ysb{j}_{g}")
                        dst = yt[:, 0:GW]
                    else:
                        if half == 0:
                            ysb[j] = work.tile([128, 2 * GW], BF16,
                                               tag=f"ysb{j}", bufs=2,
                                               name=f"ysb{j}_{pair}")
                        yt = ysb[j]
                        dst = yt[:, half * GW:(half + 1) * GW]
                    # y evictions on vector (per-partition bias add + cast)
                    nc.vector.tensor_scalar_add(dst, yp[:, :],
                                                bias_sb[:, j:j + 1])
                    # stores: j0/j1 on sync HWDGE, j2 on SWDGE; last two
                    # groups stored per-group so the final transfers (and
                    # their HBM write receipts) are small
                    if last2:
                        eng = (nc.sync, nc.sync, nc.gpsimd)[j]
                        eng.dma_start(
                            y_d[j, :, g * GW:(g + 1) * GW], yt[:, 0:GW])
                    elif half == 1:
                        eng = (nc.sync, nc.sync, nc.gpsimd)[j]
                        eng.dma_start(
                            y_d[j, :, pair * 2 * GW:(pair + 1) * 2 * GW],
                            yt[:, 0:2 * GW])

    nc.compile()
    return nc


_PROGRAM = None


def _get_program():
    global _PROGRAM
    if _PROGRAM is None:
        _PROGRAM = build_program()
    return _PROGRAM


def _host_prep(x, Wv, qk, Wo, bo):
    x = np.asarray(x, dtype=np.float32)
    xc = x.reshape(N_CORES, BPC, NPOS, DIM)
    xu0 = np.ascontiguousarray(xc[:, :, 0:U0, :]).astype(NP_BF16)
    xu1 = np.ascontiguousarray(xc[:, :, U0:NPOS, :]).astype(NP_BF16)
    # W = Wv @ Wo folded once, tiled [128, 9*128] with block k*3+j
    W = (np.asarray(Wv, np.float32) @ np.asarray(Wo, np.float32))
    wt = np.ascontiguousarray(
        W.reshape(3, 128, 3, 128).transpose(1, 0, 2, 3).reshape(128, 9 * 128)
    ).astype(NP_BF16)
    # MT[(u,v),(p,q)] = qk[13+u-p, 13+v-q]
    qk2 = np.asarray(qk, np.float32).reshape(KS, KS)
    idx = (KS // 2) + np.arange(IMG)[:, None] - np.arange(IMG)[None, :]
    MT = qk2[idx[:, None, :, None], idx[None, :, None, :]].reshape(NPOS, NPOS)
    mt = np.zeros((U0, GW), np.float32)
    mt[:, 0:NPOS] = MT[0:U0, :]
    mt[0:U1, NPOS:GW] = MT[U0:NPOS, :]
    mt = mt.astype(NP_BF16)
    bias = np.zeros((128, 3), np.float32)
    bias[:, 0] = np.asarray(bo, np.float32)[0:128]
    bias[:, 1] = np.asarray(bo, np.float32)[128:256]
    bias[:, 2] = np.asarray(bo, np.float32)[256:384]
    return xu0, xu1, wt, mt, bias


def _unpack_core(y2):
    # y2: [3, 128, BPC*NPOS] bf16 -> (BPC, NPOS, DIM) fp32
    return np.ascontiguousarray(
        np.asarray(y2).reshape(3, 128, BPC, NPOS).transpose(2, 3, 0, 1)
        .reshape(BPC, NPOS, DIM)
    ).astype(np.float32)


def _run(x, Wv, qk, Wo, bo, **spmd_kwargs):
    xu0, xu1, wt, mt, bias = _host_prep(x, Wv, qk, Wo, bo)
    nc = _get_program()
    in_maps = [
        {"xu0": xu0[c], "xu1": xu1[c], "w": wt, "mt": mt, "bias": bias}
        for c in range(N_CORES)
    ]
    res = run_bass_kernel_spmd(nc, in_maps, list(range(N_CORES)), **spmd_kwargs)
    y = np.concatenate(
        [_unpack_core(res.results[c]["y"]) for c in range(N_CORES)], axis=0)
    return y, res


def kernel(x, Wv, qk, Wo, bo):
    y, _ = _run(x, Wv, qk, Wo, bo)
    return y


# revision 6
# speedup vs baseline: 1.3501x; 1.2311x over previous
# Trainium2 Bass kernel for nn_Attention_88313117540497.
#
# Reference computation (per batch b of 128):
#   v = x_b @ Wv                      (196, 384) @ (384, 512)
#   conv: each of the 512 channels' 14x14 image convolved with a 27x27
#         kernel qk at padding 13 -> same 14x14 output
#   y = conv_out @ Wo + bo            (196, 512) @ (512, 384)
#
# Math restructuring:
#  1. The 27x27 kernel at padding 13 covers every input pixel for every
#     output pixel, so the conv is a dense 196x196 linear map M over
#     positions, shared by all batches/channels: conv == matmul.
#  2. Folding W = Wv @ Wo (384x384) removes INNER=512:
#     y_b = (M @ x_b) @ W + bo.
#  3. All-transposed, M-first dataflow minimizes PE streaming cycles:
#       Z.T[d,p] = sum_u X[u,d] MT[u,p]   (lhsT = X chunk, rhs = MT)
#       Y.T[e,p] = sum_d W[d,e] Z.T[d,p]  (lhsT = W tile,  rhs = Z.T)
#     2940 PE cycles/batch vs 3840 for the W-first token-major order.
#  4. bf16 everywhere (4.4e-3 max-normalized error vs the 2e-2 budget):
#     1 cycle/row at any N (f32r is 4 cyc/row under N=256), single fast
#     LDWEIGHTS per matmul (fp32-family needs a LOW/HIGH pair - the
#     baseline's PE throttle), half the HBM bytes both directions.
#
# Schedule (from trace analysis of the first bf16 cut):
#  - x is host-packed feature-transposed and token-PADDED to 256 so every
#    load is a single 128-partition DMA with one contiguous slab per
#    partition (the unpadded 68-row token tail otherwise needs its own
#    slow scattered transfers; zero rows contribute nothing to the
#    contraction).
#  - MT rides the fast sync HWDGE ring FIRST (it gates every stage-1
#    matmul; on SWDGE it landed at 12.2us and stalled the PE 3.8us),
#    W on the scalar HWDGE ring, bias on SWDGE.
#  - ZT/YT are software-pipelined one group apart in emission order so
#    the PE never waits for the scalar-engine PSUM evictions of the
#    group it just computed (PE streams are executed in order).
#  - PSUM evictions: stage-1 on scalar (ACT copy+cast), stage-2 on
#    vector (tensor_scalar bias-add+cast), ~587/619ns per 392-elem op,
#    each engine ~14.5us < PE ~21us.
#  - y stores: 2-group grains on sync/gpsimd mid-kernel; the last two
#    groups go per-group on the two HWDGE rings only (a final SWDGE
#    store costs ~2.5us of Q7 descriptor tail).
#
# Sharding: data-parallel over batch, 16 batches per core, weights
# replicated. No collectives.

import numpy as np
import ml_dtypes

import concourse.bass as bass
from concourse import bacc
import concourse.mybir as mybir
import concourse.tile as tile
from concourse.bass_utils import run_bass_kernel_spmd

N_CORES = 8
B = 128                 # total batch
BPC = B // N_CORES      # batches per core
DIM = 384
NPOS = 196              # 14*14 positions
IMG = 14
KS = 27                 # conv kernel size
TPAD = 256              # tokens padded to 2 full partition chunks
U1 = NPOS - 128         # valid rows in token chunk 1 (68)

F32 = mybir.dt.float32
BF16 = mybir.dt.bfloat16
NP_BF16 = ml_dtypes.bfloat16

NG = BPC // 2           # 2-batch compute groups
GW = 2 * NPOS           # output cols per group: 392
BW = 2 * DIM            # packed x cols per batch: 768
XGROUPS = [(0, 2), (2, 2), (4, 4), (8, 8)]   # (start batch, count)
NXG = len(XGROUPS)


def build_program():
    nc = bacc.Bacc("TRN2", debug=False)

    # x packed: xp[p, b*768 + c*384 + d] = x[b, 128c+p, d] (0 for pad rows)
    xp_d = nc.dram_tensor("xp", [128, BPC * BW], BF16, kind="ExternalInput")
    # MT packed: cols 0:196 = MT[0:128,:]; cols 196:392 = MT[128:196,:]
    # on rows 0:68, zeros on rows 68:128 (pad tokens)
    mt_d = nc.dram_tensor("mt", [128, GW], BF16, kind="ExternalInput")
    # W folded, tiled: block k*3+j = W[128k:128k+128, 128j:128j+128]
    w_d = nc.dram_tensor("w", [128, 9 * 128], BF16, kind="ExternalInput")
    bias_d = nc.dram_tensor("bias", [128, 3], F32, kind="ExternalInput")
    # y transposed: [e-chunk, e%128, batch-token stream]
    y_d = nc.dram_tensor("y", [3, 128, BPC * NPOS], BF16, kind="ExternalOutput")

    with tile.TileContext(nc) as tc:
        with (
            tc.tile_pool(name="const", bufs=1) as const,
            tc.tile_pool(name="work", bufs=2) as work,
            tc.tile_pool(name="psum", bufs=2, space="PSUM") as psum,
        ):
            # ---- constants: mt gates everything -> first on sync ----
            mt_sb = const.tile([128, GW], BF16)
            nc.sync.dma_start(mt_sb[:, :], mt_d[:, :])
            w_sb = const.tile([128, 9 * 128], BF16)
            nc.scalar.dma_start(w_sb[:, :], w_d[:, :])
            bias_sb = const.tile([128, 3], F32)
            nc.gpsimd.dma_start(bias_sb[:, :], bias_d[:, :])

            # ---- x loads: one contiguous 128-partition DMA per group ----
            xp_t = {}
            for gi, (s, nb) in enumerate(XGROUPS):
                t = work.tile([128, nb * BW], BF16, tag="xp", bufs=NXG,
                              name=f"xp{gi}")
                nc.sync.dma_start(t[:, 0:nb * BW],
                                  xp_d[:, s * BW:(s + nb) * BW])
                for b in range(s, s + nb):
                    xp_t[b] = (t, (b - s) * BW)

            # ---- PE warm-up on framework const APs: ramp the HAM clock
            # while the first data is in flight ----
            warm_c = nc.const_aps.tensor(1.0, (128, 1))
            for wi in range(12):
                warm = psum.tile([128, GW], F32, tag="z0", name=f"warm{wi}")
                nc.tensor.matmul(
                    warm[0:1, 0:1], lhsT=warm_c, rhs=warm_c,
                    start=True, stop=True,
                )

            # ---- main loop: ZT(g) emitted one group ahead of YT(g) ----
            ZBUFS = [2, 2, 1]
            zsb_g = {}

            def emit_zt(g):
                ba, bb = 2 * g, 2 * g + 1
                zsb = []
                for k in range(3):
                    zp = psum.tile([128, GW], F32, tag=f"z{k}",
                                   bufs=ZBUFS[k], name=f"zp{k}_{g}")
                    for half, b in ((0, ba), (1, bb)):
                        t, off = xp_t[b]
                        c0 = half * NPOS
                        nc.tensor.matmul(
                            zp[:, c0:c0 + NPOS],
                            lhsT=t[:, off + k * 128:off + (k + 1) * 128],
                            rhs=mt_sb[:, 0:NPOS],
                            start=True, stop=False,
                        )
                        nc.tensor.matmul(
                            zp[:, c0:c0 + NPOS],
                            lhsT=t[:, off + DIM + k * 128:
                                   off + DIM + (k + 1) * 128],
                            rhs=mt_sb[:, NPOS:GW],
                            start=False, stop=True,
                        )
                    z = work.tile([128, GW], BF16, tag=f"zsb{k}", bufs=2,
                                  name=f"zsb{k}_{g}")
                    # stage-1 evictions on scalar (ACT copy + cast)
                    nc.scalar.copy(z[:, :], zp[:, :])
                    zsb.append(z)
                zsb_g[g] = zsb

            ysb = {}

            def emit_yt(g):
                zsb = zsb_g.pop(g)
                pair, half = g // 2, g % 2
                last2 = g >= NG - 2
                for j in range(3):
                    yp = psum.tile([128, GW], F32, tag=f"y{j}", bufs=1,
                                   name=f"yp{j}_{g}")
                    for k in range(3):
                        nc.tensor.matmul(
                            yp[:, :],
                            lhsT=w_sb[:, (k * 3 + j) * 128:
                                      (k * 3 + j + 1) * 128],
                            rhs=zsb[k][:, :],
                            start=(k == 0), stop=(k == 2),
                        )
                    if last2:
                        yt = work.tile([128, GW], BF16, tag=f"ysb{j}", bufs=2,
                                       name=f"ysb{j}_{g}")
                        dst = yt[:, 0:GW]
                    else:
                        if half == 0:
                            ysb[j] = work.tile([128, 2 * GW], BF16,
                                               tag=f"ysb{j}", bufs=2,
                                               name=f"ysb{j}_{pair}")
                        yt = ysb[j]
                        dst = yt[:, half * GW:(half + 1) * GW]
                    # stage-2 evictions on vector (bias add + cast)
                    nc.vector.tensor_scalar_add(dst, yp[:, :],
                                                bias_sb[:, j:j + 1])
                    if last2:
                        # small final transfers, HWDGE rings only
                        eng = (nc.sync, nc.scalar,
                               nc.sync if g == NG - 2 else nc.scalar)[j]
                        eng.dma_start(
                            y_d[j, :, g * GW:(g + 1) * GW], yt[:, 0:GW])
                    elif half == 1:
                        eng = (nc.sync, nc.sync, nc.gpsimd)[j]
                        eng.dma_start(
                            y_d[j, :, pair * 2 * GW:(pair + 1) * 2 * GW],
                            yt[:, 0:2 * GW])

            emit_zt(0)
            for g in range(NG):
                if g + 1 < NG:
                    emit_zt(g + 1)
                emit_yt(g)

    nc.compile()
    return nc


_PROGRAM = None


def _get_program():
    global _PROGRAM
    if _PROGRAM is None:
        _PROGRAM = build_program()
    return _PROGRAM


def _host_prep(x, Wv, qk, Wo, bo):
    x = np.asarray(x, dtype=np.float32)
    xc = x.reshape(N_CORES, BPC, NPOS, DIM)
    # xp[core, p, b, c, d] = x[core, b, 128c+p, d], pad rows zero
    xpad = np.zeros((N_CORES, BPC, 2, 128, DIM), np.float32)
    xpad[:, :, 0, :, :] = xc[:, :, 0:128, :]
    xpad[:, :, 1, 0:U1, :] = xc[:, :, 128:NPOS, :]
    xp = np.ascontiguousarray(
        xpad.transpose(0, 3, 1, 2, 4).reshape(N_CORES, 128, BPC * BW)
    ).astype(NP_BF16)
    # W = Wv @ Wo folded once, tiled [128, 9*128] with block k*3+j
    W = (np.asarray(Wv, np.float32) @ np.asarray(Wo, np.float32))
    wt = np.ascontiguousarray(
        W.reshape(3, 128, 3, 128).transpose(1, 0, 2, 3).reshape(128, 9 * 128)
    ).astype(NP_BF16)
    # MT[(u,v),(p,q)] = qk[13+u-p, 13+v-q]
    qk2 = np.asarray(qk, np.float32).reshape(KS, KS)
    idx = (KS // 2) + np.arange(IMG)[:, None] - np.arange(IMG)[None, :]
    MT = qk2[idx[:, None, :, None], idx[None, :, None, :]].reshape(NPOS, NPOS)
    mt = np.zeros((128, GW), np.float32)
    mt[:, 0:NPOS] = MT[0:128, :]
    mt[0:U1, NPOS:GW] = MT[128:NPOS, :]
    mt = mt.astype(NP_BF16)
    bias = np.zeros((128, 3), np.float32)
    bias[:, 0] = np.asarray(bo, np.float32)[0:128]
    bias[:, 1] = np.asarray(bo, np.float32)[128:256]
    bias[:, 2] = np.asarray(bo, np.float32)[256:384]
    return xp, wt, mt, bias


def _unpack_core(y2):
    # y2: [3, 128, BPC*NPOS] bf16 -> (BPC, NPOS, DIM) fp32
    return np.ascontiguousarray(
        np.asarray(y2).reshape(3, 128, BPC, NPOS).transpose(2, 3, 0, 1)
        .reshape(BPC, NPOS, DIM)
    ).astype(np.float32)


def _run(x, Wv, qk, Wo, bo, **spmd_kwargs):
    xp, wt, mt, bias = _host_prep(x, Wv, qk, Wo, bo)
    nc = _get_program()
    in_maps = [
        {"xp": xp[c], "w": wt, "mt": mt, "bias": bias}
        for c in range(N_CORES)
    ]
    res = run_bass_kernel_spmd(nc, in_maps, list(range(N_CORES)), **spmd_kwargs)
    y = np.concatenate(
        [_unpack_core(res.results[c]["y"]) for c in range(N_CORES)], axis=0)
    return y, res


def kernel(x, Wv, qk, Wo, bo):
    y, _ = _run(x, Wv, qk, Wo, bo)
    return y
